# revision 16
# baseline (speedup 1.0000x reference)
"""Trainium2 Bass kernel for nn_ELM_AE_FatSpectral_Ensemble.

Data-parallel over batch: 4 samples/core on 8 cores. Per (sample, member):
  XrT[w,c] = RfullT.T @ xT   (bilinear+antialias resize via PE matmul)
  XnT      = zscore(XrT) along c (bn_stats + per-partition scale on free dim)
  H        = sigmoid(W @ Xn);  G = H H^T (16x16)
  G^-1 via Newton-Schulz on block-diagonal [128,128] supermatrices
  var(B), B = C G^-1, C = Xn H^T:
     quad_i = C_i G^-2 C_i^T, t_i = C_i (G^-1 1)
     out_i = quad_i/(Q-1) - t_i^2/(Q(Q-1))
Blocks of the supermatrix: b = 4*si + m (si = sample within pair), one
supermatrix per sample pair (spr), built super-outer so Newton-Schulz of
super 0 overlaps phase 1 of super 1.

V3: inputs are transposed on the host ([S, uv, c] layout) so the PE never
transposes raw x; DMA-cast fp32->fp16 lands directly in the resize rhs
layout at line rate. Heavy path all fp16 (PE 1 cycle/row + FWL); PSUM
accumulation fp32; Newton-Schulz fp32.
"""

import numpy as np

import concourse.bacc as bacc
import concourse.tile as tile
from concourse import mybir
from concourse.bass_utils import run_bass_kernel_spmd

F32 = mybir.dt.float32
F16 = mybir.dt.float16
AF = mybir.ActivationFunctionType
ALU = mybir.AluOpType

S = 4
NCORES = 8
SP = 14
WH = SP * SP
Q = 16
MEMBERS = [(256, 56), (512, 28), (1024, 14), (2048, 7)]
OFFS = [0, 256, 768, 1792]
DTOT = 3840
NEWTON_ITERS = 10
MORDER = [3, 2, 1, 0]     # small members first: shortens the startup DMA stall


def _weight_mat(n_in, n_out):
    scale = n_out / n_in
    kernel_scale = max(1.0, 1.0 / scale)
    sample_f = (np.arange(n_out) + 0.5) / scale - 0.5
    x = np.abs(sample_f[:, None] - np.arange(n_in)[None, :]) / kernel_scale
    w = np.maximum(0.0, 1.0 - x)
    total = w.sum(axis=1, keepdims=True)
    return (w / np.where(total > 0, total, 1)).astype(np.float32)


def _consts():
    rts = {}
    for m, (c, sp) in enumerate(MEMBERS):
        if sp == SP:
            continue
        R = _weight_mat(sp, SP)
        rt = np.kron(R, R).T.astype(np.float32)   # [uv, 196]
        pad = (-rt.shape[0]) % 128
        if pad:
            rt = np.concatenate([rt, np.zeros((pad, WH), np.float32)], 0)
        rts[m] = np.ascontiguousarray(rt)
    ident = np.eye(128, dtype=np.float32)
    p16 = np.kron(np.eye(8, dtype=np.float32), np.ones((16, 16), np.float32))
    p16 = p16.astype(np.float32)
    # mask8[m, spr][p, col]: per-super sample masks
    mask8 = np.zeros((4, 2, 128, 4), np.float32)
    for m in range(4):
        for spr in range(2):
            for si in range(2):
                b = 4 * si + m
                mask8[m, spr, 16 * b:16 * b + 16, 2 * spr + si] = 1.0
    mask8 = np.ascontiguousarray(mask8.transpose(2, 0, 1, 3))  # [128, 4, 2, 4]
    return rts, ident, p16, mask8


def _chunks(n, sz=128):
    return [(i, min(sz, n - i)) for i in range(0, n, sz)]


def _build_program(debug=False):
    rts, ident_np, p16_np, mask8_np = _consts()

    nc = bacc.Bacc()
    xin, wt, rtd = {}, {}, {}
    for m, (c, sp) in enumerate(MEMBERS):
        # host-transposed fp16 input: [S, uv, c]
        xin[m] = nc.dram_tensor(f"x{m}", [S, sp * sp, c], F16,
                                kind="ExternalInput")
        wt[m] = nc.dram_tensor(f"wt{m}", [c, Q], F32, kind="ExternalInput")
        if m in rts:
            rtd[m] = nc.dram_tensor(f"rt{m}", list(rts[m].shape), F32,
                                    kind="ExternalInput")
    identd = nc.dram_tensor("ident", [128, 128], F32, kind="ExternalInput")
    p16d = nc.dram_tensor("p16", [128, 128], F32, kind="ExternalInput")
    mask8d = nc.dram_tensor("mask8", [128, 4, 2, 4], F32, kind="ExternalInput")
    outd = nc.dram_tensor("out", [S, DTOT], F32, kind="ExternalOutput")

    nzk = {}
    for m in rts:
        uv = MEMBERS[m][1] ** 2
        nzk[m] = {}
        for Mi, (Mo, Msz) in enumerate([(0, 128), (128, 68)]):
            nzk[m][Mi] = [ki for ki, (ko, ksz) in enumerate(_chunks(uv))
                          if np.any(rts[m][ko:ko + ksz, Mo:Mo + Msz] != 0)]

    from contextlib import ExitStack
    _ceng = [0]

    def _pcopy(out, in_):
        _ceng[0] ^= 1
        (nc.scalar.copy if _ceng[0] else nc.vector.tensor_copy)(out=out, in_=in_)

    with tile.TileContext(nc) as tc, ExitStack() as _es:
        _p = lambda **kw: _es.enter_context(tc.tile_pool(**kw))
        consts = _p(name="consts", bufs=1)
        xtp = _p(name="xtp", bufs=2)
        xntp = _p(name="xntp", bufs=2)
        xnp = _p(name="xnp", bufs=3)
        hp = _p(name="hp", bufs=2)
        htp = _p(name="htp", bufs=2)
        sup = _p(name="sup", bufs=1)
        newt = _p(name="newt", bufs=2)
        smalls = _p(name="smalls", bufs=4)
        psbp = _p(name="psbp", bufs=2)
        tsqp = _p(name="tsqp", bufs=2)
        outp = _p(name="outp", bufs=2)
        # PSUM budget (16KB/partition): pt 2x1KB + big512 3x2KB + ns 2x0.5KB
        # + hq 3x2KB = 15KB
        pt = _p(name="pt", bufs=2, space="PSUM")
        pacc = _p(name="pacc", bufs=3, space="PSUM")
        nsp = _p(name="nsp", bufs=1, space="PSUM")
        hq = _p(name="hq", bufs=2, space="PSUM")
        if True:
            # ---------- constants ----------
            ident_sb = consts.tile([128, 128], F32, tag="ident")
            nc.gpsimd.dma_start(out=ident_sb, in_=identd[:, :])
            identh_sb = consts.tile([128, 128], F16, tag="identh")
            nc.gpsimd.dma_start(out=identh_sb, in_=identd[:, :])
            p16_sb = consts.tile([128, 128], F32, tag="p16")
            nc.gpsimd.dma_start(out=p16_sb, in_=p16d[:, :])
            mask8_sb = consts.tile([128, 4, 2, 4], F32, tag="mask8")
            nc.gpsimd.dma_start(out=mask8_sb, in_=mask8d[:, :, :, :])
            oq8_sb = consts.tile([128, 4, 2, 4], F16, tag="oq8")
            nc.vector.tensor_scalar_mul(
                oq8_sb.rearrange("p a b c -> p (a b c)"),
                mask8_sb.rearrange("p a b c -> p (a b c)"), 1.0 / (Q - 1))
            ones_sb = consts.tile([128, 1], F32, tag="ones")
            nc.vector.memset(ones_sb, 1.0)
            zer_sb = consts.tile([128, 256], F16, tag="zer")
            nc.vector.memset(zer_sb, 0.0)

            rtt, wtt = {}, {}
            for m, (c, sp) in enumerate(MEMBERS):
                cc_n = c // 128
                wtt[m] = consts.tile([128, cc_n, Q], F16, tag=f"wt{m}",
                                     name=f"wt{m}")
                nc.gpsimd.dma_start(
                    out=wtt[m],
                    in_=wt[m][:, :].rearrange("(k p) q -> p k q", p=128))
                if m in rts:
                    nk = rts[m].shape[0] // 128
                    rtt[m] = consts.tile([128, nk, WH], F16, tag=f"rt{m}",
                                         name=f"rt{m}")
                    nc.gpsimd.dma_start(
                        out=rtt[m],
                        in_=rtd[m][:, :].rearrange("(k p) w -> p k w", p=128))

            ct_all, gsup = [], []
            for spr in range(2):
                t = sup.tile([128, DTOT], F16, tag=f"ct{spr}", name=f"ct{spr}")
                ct_all.append(t)
                g = sup.tile([128, 128], F32, tag=f"gsup{spr}",
                             name=f"gsup{spr}")
                nc.vector.memset(g, 0.0)
                gsup.append(g)

            m2_sb = [None, None]
            r_sb = [None, None]

            # ============ phase 1+2, super-outer =============
            for spr in range(2):
                for m in MORDER:
                    c, sp = MEMBERS[m]
                    uv = sp * sp
                    cc_n = c // 128
                    kch = _chunks(uv)
                    nch = _chunks(c, 512)

                    xnt0, xnt1 = {}, {}
                    for si in range(2):
                        s = 2 * spr + si
                        x0t = xntp.tile([128, c], F16, tag=f"xnt0_{si}",
                                        name=f"xnt0_{si}")
                        x1t = xntp.tile([68, c], F16, tag=f"xnt1_{si}",
                                        name=f"xnt1_{si}")
                        xnt0[si], xnt1[si] = x0t, x1t

                        if m == 2:
                            # identity resize: XrT comes straight from DRAM
                            nc.sync.dma_start(out=x0t,
                                              in_=xin[m][s, 0:128, :])
                            nc.sync.dma_start(out=x1t[:68, :],
                                              in_=xin[m][s, 128:196, :])
                        else:
                            # xt = x^T [uv(part), c] fp16, direct DMA-cast
                            xt = xtp.tile([128, len(kch), c], F16,
                                          tag=f"xt{m}", name=f"xt{m}")
                            nfull = uv // 128
                            rem = uv - nfull * 128
                            if nfull:
                                nc.sync.dma_start(
                                    out=xt[:, :nfull, :],
                                    in_=xin[m][s, 0:nfull * 128, :].rearrange(
                                        "(k p) c -> p k c", p=128))
                            if rem:
                                nc.sync.dma_start(
                                    out=xt[:rem, nfull, :],
                                    in_=xin[m][s, nfull * 128:uv, :])
                            for ni, (no, nsz) in enumerate(nch):
                                for Mo, Msz, dst in ((0, 128, x0t),
                                                     (128, 68, x1t)):
                                    pr = pacc.tile([128, 512], F32, tag="pacc")
                                    ks = nzk[m][0 if Mo == 0 else 1]
                                    for idx, ki in enumerate(ks):
                                        ko, ksz = kch[ki]
                                        nc.tensor.matmul(
                                            pr[:Msz, :nsz],
                                            lhsT=rtt[m][:ksz, ki, Mo:Mo + Msz],
                                            rhs=xt[:ksz, ki, no:no + nsz],
                                            start=(idx == 0),
                                            stop=(idx == len(ks) - 1))
                                    _pcopy(out=dst[:Msz, no:no + nsz],
                                           in_=pr[:Msz, :nsz])

                        # zscore along free dim
                        for tl, psz in ((x0t, 128), (x1t, 68)):
                            gs = _chunks(c, 512)
                            st = smalls.tile([128, len(gs), 6], F32, tag="bnst")
                            for gi, (go, gln) in enumerate(gs):
                                nc.vector.bn_stats(out=st[:psz, gi, :],
                                                   in_=tl[:psz, go:go + gln])
                            mv = smalls.tile([128, 2], F32, tag="mv")
                            nc.vector.bn_aggr(out=mv[:psz], in_=st[:psz])
                            sd = smalls.tile([128, 1], F32, tag="sd")
                            nc.scalar.activation(out=sd[:psz], in_=mv[:psz, 1:2],
                                                 func=AF.Sqrt,
                                                 scale=c / (c - 1.0))
                            sc = smalls.tile([128, 1], F32, tag="sc")
                            nc.vector.reciprocal(out=sc[:psz], in_=sd[:psz])
                            bi = smalls.tile([128, 1], F32, tag="bi")
                            nc.vector.tensor_scalar(
                                out=bi[:psz], in0=mv[:psz, 0:1],
                                scalar1=sc[:psz], scalar2=-1.0,
                                op0=ALU.mult, op1=ALU.mult)
                            nc.gpsimd.tensor_scalar(
                                out=tl[:psz], in0=tl[:psz],
                                scalar1=sc[:psz], scalar2=bi[:psz],
                                op0=ALU.mult, op1=ALU.add)

                    # ---- H = sigmoid(W @ Xn) over the sample pair ----
                    hps = hq.tile([16, 2 * WH], F32, tag="hq")
                    for c2 in range(0, cc_n, 2):
                        cn = min(2, cc_n - c2)
                        xn = xnp.tile([128, 2, 2 * WH], F16, tag="xn")
                        for ci in range(cn):
                            ps = pt.tile([128, 512], F16, tag="pt")
                            cc = c2 + ci
                            for si in range(2):
                                off = si * WH
                                nc.tensor.transpose(
                                    ps[:128, off:off + 128],
                                    xnt0[si][:, cc * 128:(cc + 1) * 128],
                                    identh_sb)
                                nc.tensor.transpose(
                                    ps[:128, off + 128:off + 196],
                                    xnt1[si][:68, cc * 128:(cc + 1) * 128],
                                    identh_sb[:68, :68])
                            _pcopy(out=xn[:, ci, :], in_=ps[:, 0:2 * WH])
                        for ci in range(cn):
                            cc = c2 + ci
                            nc.tensor.matmul(
                                hps, lhsT=wtt[m][:, cc, :],
                                rhs=xn[:, ci, :],
                                start=(cc == 0), stop=(cc == cc_n - 1))
                    hsb = hp.tile([16, 2, WH], F16, tag="hsb")
                    nc.scalar.activation(
                        out=hsb.rearrange("p a b -> p (a b)"),
                        in_=hps, func=AF.Sigmoid)

                    htbig = {}
                    for si in range(2):
                        b = 4 * si + m
                        bo = 16 * b
                        htb = htp.tile([128, 2, 128], F16, tag="htbig")
                        nc.gpsimd.tensor_copy(
                            out=htb.rearrange("p a b -> p (a b)"),
                            in_=zer_sb)
                        pht = pt.tile([128, 512], F16, tag="pt")
                        nc.tensor.transpose(pht[:128, 0:Q],
                                            hsb[:Q, si, 0:128],
                                            identh_sb[:Q, :Q])
                        nc.tensor.transpose(pht[:68, Q:2 * Q],
                                            hsb[:Q, si, 128:196],
                                            identh_sb[:Q, :Q])
                        nc.vector.tensor_copy(
                            out=htb[:, 0, bo:bo + Q],
                            in_=pht[:128, 0:Q])
                        nc.vector.tensor_copy(
                            out=htb[:68, 1, bo:bo + Q],
                            in_=pht[:68, Q:2 * Q])
                        htbig[si] = htb
                        # G block accumulation into SBUF accumulator
                        gp = nsp.tile([128, 128], F32, tag="ns")
                        for wi, wsz in ((0, 128), (1, 68)):
                            nc.tensor.matmul(gp,
                                             lhsT=htb[:wsz, wi, :],
                                             rhs=htb[:wsz, wi, :],
                                             start=(wi == 0), stop=(wi == 1))
                        nc.vector.tensor_tensor(out=gsup[spr], in0=gsup[spr],
                                                in1=gp, op=ALU.add)
                    # C^T for both blocks of the pair
                    for ni, (no, nsz) in enumerate(nch):
                        cps = pacc.tile([128, 512], F32, tag="pacc")
                        idx = 0
                        for si in range(2):
                            for wi, wsz, xs in ((0, 128, xnt0[si]),
                                                (1, 68, xnt1[si])):
                                nc.tensor.matmul(
                                    cps[:, :nsz],
                                    lhsT=htbig[si][:wsz, wi, :],
                                    rhs=xs[:wsz, no:no + nsz],
                                    start=(idx == 0), stop=(idx == 3))
                                idx += 1
                        _pcopy(out=ct_all[spr][:, OFFS[m] + no:OFFS[m] + no +
                                               nsz],
                               in_=cps[:, :nsz])

                # ---- phase 2 for this super: Newton-Schulz ----
                # fp16 iterations (PE 1 cyc/row) + one fp32 polish iteration;
                # the polish contracts fp16 iteration error quadratically.
                g = gsup[spr]
                gh = g
                sq = newt.tile([128, 128], F32, tag="sq")
                nc.vector.tensor_mul(sq, g, g)
                rs = newt.tile([128, 1], F32, tag="rs")
                nc.vector.tensor_reduce(out=rs, in_=sq,
                                        axis=mybir.AxisListType.X, op=ALU.add)
                bps = nsp.tile([128, 128], F32, tag="ns")
                nc.tensor.matmul(bps[:128, 0:1], lhsT=p16_sb, rhs=rs,
                                 start=True, stop=True)
                bf = newt.tile([128, 1], F32, tag="bf")
                nc.scalar.activation(out=bf, in_=bps[:128, 0:1], func=AF.Sqrt)
                al = newt.tile([128, 1], F32, tag="al")
                nc.vector.reciprocal(out=al, in_=bf)
                xh = newt.tile([128, 128], F32, tag="xh")
                nc.vector.tensor_scalar_mul(xh, ident_sb, al)
                xps = None
                for it in range(NEWTON_ITERS):
                    yps = nsp.tile([128, 128], F32, tag="ns")
                    nc.tensor.matmul(yps, lhsT=gh, rhs=xh,
                                     start=True, stop=True)
                    zh = newt.tile([128, 128], F32, tag="zh")
                    nc.vector.scalar_tensor_tensor(
                        out=zh, in0=ident_sb, scalar=2.0,
                        in1=yps, op0=ALU.mult, op1=ALU.subtract)
                    xps = nsp.tile([128, 128], F32, tag="ns")
                    nc.tensor.matmul(xps, lhsT=xh, rhs=zh,
                                     start=True, stop=True)
                    if it < NEWTON_ITERS - 1:
                        xh = newt.tile([128, 128], F32, tag="xh")
                        nc.scalar.copy(out=xh, in_=xps)
                x32 = newt.tile([128, 128], F32, tag="x32")
                nc.scalar.copy(out=x32, in_=xps)
                yp2 = nsp.tile([128, 128], F32, tag="ns")
                nc.tensor.matmul(yp2, lhsT=g, rhs=x32, start=True, stop=True)
                z32 = newt.tile([128, 128], F32, tag="z32")
                nc.vector.scalar_tensor_tensor(
                    out=z32, in0=ident_sb, scalar=2.0,
                    in1=yp2, op0=ALU.mult, op1=ALU.subtract)
                xp2 = nsp.tile([128, 128], F32, tag="ns")
                nc.tensor.matmul(xp2, lhsT=x32, rhs=z32, start=True, stop=True)
                xf = newt.tile([128, 128], F32, tag="xf")
                nc.scalar.copy(out=xf, in_=xp2)
                mps = nsp.tile([128, 128], F32, tag="ns")
                nc.tensor.matmul(mps, lhsT=xf, rhs=xf,
                                 start=True, stop=True)
                m2t = sup.tile([128, 128], F16, tag=f"m2_{spr}",
                               name=f"m2_{spr}")
                nc.vector.tensor_copy(out=m2t, in_=mps)
                m2_sb[spr] = m2t
                rps = nsp.tile([128, 128], F32, tag="ns")
                nc.tensor.matmul(rps[:128, 0:1], lhsT=xf, rhs=ones_sb,
                                 start=True, stop=True)
                rt_ = sup.tile([128, 1], F32, tag=f"r_{spr}", name=f"r_{spr}")
                nc.vector.tensor_copy(out=rt_, in_=rps[:128, 0:1])
                r_sb[spr] = rt_

            # r4[m][spr] = mask8[m, spr] * r_super(spr)
            r4 = []
            for m in range(4):
                t = smalls.tile([128, 2, 4], F16, tag="r4", name=f"r4_{m}")
                nc.vector.tensor_scalar_mul(t[:, 0, :],
                                            mask8_sb[:, m, 0, :], r_sb[0])
                nc.vector.tensor_scalar_mul(t[:, 1, :],
                                            mask8_sb[:, m, 1, :], r_sb[1])
                r4.append(t)

            # ================= phase 3: variance readout =================
            for m, (c, sp) in enumerate(MEMBERS):
                for ni, (no, nsz) in enumerate(_chunks(c, 512)):
                    g0 = OFFS[m] + no
                    psbs = []
                    for spr in range(2):
                        dfp = pacc.tile([128, 512], F32, tag="pacc")
                        nc.tensor.matmul(
                            dfp[:, :nsz], lhsT=m2_sb[spr],
                            rhs=ct_all[spr][:, g0:g0 + nsz],
                            start=True, stop=True)
                        psb = psbp.tile([128, 512], F16, tag="psb")
                        nc.vector.tensor_mul(psb[:, :nsz],
                                             ct_all[spr][:, g0:g0 + nsz],
                                             dfp[:, :nsz])
                        psbs.append(psb)
                    qps = hq.tile([4, 512], F32, tag="hq")
                    tps = hq.tile([4, 512], F32, tag="hq")
                    for spr in range(2):
                        nc.tensor.matmul(
                            qps[:, :nsz],
                            lhsT=oq8_sb[:, m, spr, :],
                            rhs=psbs[spr][:, :nsz],
                            start=(spr == 0), stop=(spr == 1))
                        nc.tensor.matmul(
                            tps[:, :nsz], lhsT=r4[m][:, spr, :],
                            rhs=ct_all[spr][:, g0:g0 + nsz],
                            start=(spr == 0), stop=(spr == 1))
                    tsq = tsqp.tile([S, 512], F32, tag="tsq")
                    nc.scalar.square(out=tsq[:, :nsz], in_=tps[:S, :nsz])
                    ot = outp.tile([S, 512], F32, tag="out")
                    nc.vector.scalar_tensor_tensor(
                        out=ot[:, :nsz],
                        in0=tsq[:, :nsz], scalar=-1.0 / ((Q - 1) * Q),
                        in1=qps[:S, :nsz], op0=ALU.mult, op1=ALU.add)
                    nc.sync.dma_start(out=outd[:, g0:g0 + nsz],
                                      in_=ot[:, :nsz])

    nc.finalize()
    return nc


_CACHE = {}


def _in_maps(inputs):
    rts, ident_np, p16_np, mask8_np = _consts()
    xs = [np.asarray(inputs[f"x{m}"]) for m in range(4)]
    ws = [np.asarray(inputs[f"W{m}"]) for m in range(4)]
    in_maps = []
    for i in range(NCORES):
        im = {"ident": ident_np, "p16": p16_np, "mask8": mask8_np}
        for m, (c, sp) in enumerate(MEMBERS):
            xi = xs[m][S * i:S * (i + 1)].reshape(S, c, sp * sp)
            im[f"x{m}"] = np.ascontiguousarray(
                xi.transpose(0, 2, 1), np.float16)      # [S, uv, c] fp16
            im[f"wt{m}"] = np.ascontiguousarray(ws[m].T, np.float32)
            if m in rts:
                im[f"rt{m}"] = rts[m]
        in_maps.append(im)
    return in_maps


def kernel(x0, x1, x2, x3, W0, W1, W2, W3):
    if "nc" not in _CACHE:
        _CACHE["nc"] = _build_program()
    nc = _CACHE["nc"]
    in_maps = _in_maps(dict(x0=x0, x1=x1, x2=x2, x3=x3,
                            W0=W0, W1=W1, W2=W2, W3=W3))
    res = run_bass_kernel_spmd(nc, in_maps, list(range(NCORES)))
    return np.concatenate([r["out"] for r in res.results], axis=0)


# revision 17
# speedup vs baseline: 1.2220x; 1.2220x over previous
"""Trainium2 Bass kernel for nn_ELM_AE_FatSpectral_Ensemble.

Data-parallel over batch: 4 samples/core on 8 cores. Per (sample, member):
  XrT[w,c] = RfullT.T @ xT   (bilinear+antialias resize via PE matmul)
  XnT      = zscore(XrT) along c (bn_stats + per-partition scale on free dim)
  H        = sigmoid(W @ Xn);  G = H H^T (16x16)
  G^-1 via Newton-Schulz on block-diagonal [128,128] supermatrices
  var(B), B = C G^-1, C = Xn H^T:
     quad_i = C_i G^-2 C_i^T, t_i = C_i (G^-1 1)
     out_i = quad_i/(Q-1) - t_i^2/(Q(Q-1))
Blocks of the supermatrix: b = 4*si + m (si = sample within pair), one
supermatrix per sample pair (spr), built super-outer so Newton-Schulz of
super 0 overlaps phase 1 of super 1.

V3: inputs are transposed on the host ([S, uv, c] layout) so the PE never
transposes raw x; DMA-cast fp32->fp16 lands directly in the resize rhs
layout at line rate. Heavy path all fp16 (PE 1 cycle/row + FWL); PSUM
accumulation fp32; Newton-Schulz fp32.
"""

import numpy as np

import concourse.bacc as bacc
import concourse.tile as tile
from concourse import mybir
from concourse.bass_utils import run_bass_kernel_spmd

F32 = mybir.dt.float32
F16 = mybir.dt.float16
AF = mybir.ActivationFunctionType
ALU = mybir.AluOpType

S = 4
NCORES = 8
SP = 14
WH = SP * SP
Q = 16
MEMBERS = [(256, 56), (512, 28), (1024, 14), (2048, 7)]
OFFS = [0, 256, 768, 1792]
DTOT = 3840
NEWTON_ITERS = 10
MORDER = [3, 2, 1, 0]     # small members first: shortens the startup DMA stall


def _weight_mat(n_in, n_out):
    scale = n_out / n_in
    kernel_scale = max(1.0, 1.0 / scale)
    sample_f = (np.arange(n_out) + 0.5) / scale - 0.5
    x = np.abs(sample_f[:, None] - np.arange(n_in)[None, :]) / kernel_scale
    w = np.maximum(0.0, 1.0 - x)
    total = w.sum(axis=1, keepdims=True)
    return (w / np.where(total > 0, total, 1)).astype(np.float32)


def _consts():
    rts = {}
    for m, (c, sp) in enumerate(MEMBERS):
        if sp == SP:
            continue
        R = _weight_mat(sp, SP)
        rt = np.kron(R, R).T.astype(np.float32)   # [uv, 196]
        pad = (-rt.shape[0]) % 128
        if pad:
            rt = np.concatenate([rt, np.zeros((pad, WH), np.float32)], 0)
        rts[m] = np.ascontiguousarray(rt)
    ident = np.eye(128, dtype=np.float32)
    p16 = np.kron(np.eye(8, dtype=np.float32), np.ones((16, 16), np.float32))
    p16 = p16.astype(np.float32)
    # mask8[m, spr][p, col]: per-super sample masks
    mask8 = np.zeros((4, 2, 128, 4), np.float32)
    for m in range(4):
        for spr in range(2):
            for si in range(2):
                b = 4 * si + m
                mask8[m, spr, 16 * b:16 * b + 16, 2 * spr + si] = 1.0
    mask8 = np.ascontiguousarray(mask8.transpose(2, 0, 1, 3))  # [128, 4, 2, 4]
    return rts, ident, p16, mask8


def _chunks(n, sz=128):
    return [(i, min(sz, n - i)) for i in range(0, n, sz)]


def _build_program(debug=False):
    rts, ident_np, p16_np, mask8_np = _consts()

    nc = bacc.Bacc()
    xin, wt, rtd = {}, {}, {}
    for m, (c, sp) in enumerate(MEMBERS):
        # host-transposed fp16 input: [S, uv, c]
        xin[m] = nc.dram_tensor(f"x{m}", [S, sp * sp, c], F16,
                                kind="ExternalInput")
        wt[m] = nc.dram_tensor(f"wt{m}", [c, Q], F32, kind="ExternalInput")
        if m in rts:
            rtd[m] = nc.dram_tensor(f"rt{m}", list(rts[m].shape), F32,
                                    kind="ExternalInput")
    identd = nc.dram_tensor("ident", [128, 128], F32, kind="ExternalInput")
    p16d = nc.dram_tensor("p16", [128, 128], F32, kind="ExternalInput")
    mask8d = nc.dram_tensor("mask8", [128, 4, 2, 4], F32, kind="ExternalInput")
    outd = nc.dram_tensor("out", [S, DTOT], F32, kind="ExternalOutput")

    nzk = {}
    for m in rts:
        uv = MEMBERS[m][1] ** 2
        nzk[m] = {}
        for Mi, (Mo, Msz) in enumerate([(0, 128), (128, 68)]):
            nzk[m][Mi] = [ki for ki, (ko, ksz) in enumerate(_chunks(uv))
                          if np.any(rts[m][ko:ko + ksz, Mo:Mo + Msz] != 0)]

    from contextlib import ExitStack
    _ceng = [0]

    def _pcopy(out, in_):
        _ceng[0] ^= 1
        (nc.scalar.copy if _ceng[0] else nc.vector.tensor_copy)(out=out, in_=in_)

    with tile.TileContext(nc) as tc, ExitStack() as _es:
        _p = lambda **kw: _es.enter_context(tc.tile_pool(**kw))
        consts = _p(name="consts", bufs=1)
        xtp = _p(name="xtp", bufs=2)
        xntp = _p(name="xntp", bufs=2)
        xnp = _p(name="xnp", bufs=3)
        hp = _p(name="hp", bufs=2)
        htp = _p(name="htp", bufs=2)
        sup = _p(name="sup", bufs=1)
        newt = _p(name="newt", bufs=2)
        smalls = _p(name="smalls", bufs=4)
        psbp = _p(name="psbp", bufs=2)
        tsqp = _p(name="tsqp", bufs=2)
        outp = _p(name="outp", bufs=2)
        # PSUM budget (16KB/partition): pt 2x1KB + big512 3x2KB + ns 2x0.5KB
        # + hq 3x2KB = 15KB
        pt = _p(name="pt", bufs=2, space="PSUM")
        pacc = _p(name="pacc", bufs=3, space="PSUM")
        nsp = _p(name="nsp", bufs=1, space="PSUM")
        hq = _p(name="hq", bufs=2, space="PSUM")
        if True:
            # ---------- constants ----------
            ident_sb = consts.tile([128, 128], F32, tag="ident")
            nc.gpsimd.dma_start(out=ident_sb, in_=identd[:, :])
            identh_sb = consts.tile([128, 128], F16, tag="identh")
            nc.gpsimd.dma_start(out=identh_sb, in_=identd[:, :])
            p16_sb = consts.tile([128, 128], F32, tag="p16")
            nc.gpsimd.dma_start(out=p16_sb, in_=p16d[:, :])
            mask8_sb = consts.tile([128, 4, 2, 4], F32, tag="mask8")
            nc.gpsimd.dma_start(out=mask8_sb, in_=mask8d[:, :, :, :])
            oq8_sb = consts.tile([128, 4, 2, 4], F16, tag="oq8")
            nc.vector.tensor_scalar_mul(
                oq8_sb.rearrange("p a b c -> p (a b c)"),
                mask8_sb.rearrange("p a b c -> p (a b c)"), 1.0 / (Q - 1))
            ones_sb = consts.tile([128, 1], F32, tag="ones")
            nc.vector.memset(ones_sb, 1.0)
            zer_sb = consts.tile([128, 256], F16, tag="zer")
            nc.vector.memset(zer_sb, 0.0)

            rtt, wtt = {}, {}
            for m, (c, sp) in enumerate(MEMBERS):
                cc_n = c // 128
                wtt[m] = consts.tile([128, cc_n, Q], F16, tag=f"wt{m}",
                                     name=f"wt{m}")
                nc.gpsimd.dma_start(
                    out=wtt[m],
                    in_=wt[m][:, :].rearrange("(k p) q -> p k q", p=128))
                if m in rts:
                    nk = rts[m].shape[0] // 128
                    rtt[m] = consts.tile([128, nk, WH], F16, tag=f"rt{m}",
                                         name=f"rt{m}")
                    nc.gpsimd.dma_start(
                        out=rtt[m],
                        in_=rtd[m][:, :].rearrange("(k p) w -> p k w", p=128))

            ct_all, gsup = [], []
            for spr in range(2):
                t = sup.tile([128, DTOT], F16, tag=f"ct{spr}", name=f"ct{spr}")
                ct_all.append(t)
                g = sup.tile([128, 128], F32, tag=f"gsup{spr}",
                             name=f"gsup{spr}")
                nc.vector.memset(g, 0.0)
                gsup.append(g)

            m2_sb = [None, None]
            r_sb = [None, None]

            # ============ phase 1+2, super-outer =============
            for spr in range(2):
                for m in MORDER:
                    c, sp = MEMBERS[m]
                    uv = sp * sp
                    cc_n = c // 128
                    kch = _chunks(uv)
                    nch = _chunks(c, 512)

                    xnt0, xnt1 = {}, {}
                    for si in range(2):
                        s = 2 * spr + si
                        x0t = xntp.tile([128, c], F16, tag=f"xnt0_{si}",
                                        name=f"xnt0_{si}")
                        x1t = xntp.tile([68, c], F16, tag=f"xnt1_{si}",
                                        name=f"xnt1_{si}")
                        xnt0[si], xnt1[si] = x0t, x1t

                        if m == 2:
                            # identity resize: XrT comes straight from DRAM
                            nc.gpsimd.dma_start(out=x0t,
                                                in_=xin[m][s, 0:128, :])
                            nc.gpsimd.dma_start(out=x1t[:68, :],
                                                in_=xin[m][s, 128:196, :])
                        else:
                            # xt = x^T [uv(part), c] fp16, direct DMA-cast
                            xt = xtp.tile([128, len(kch), c], F16,
                                          tag=f"xt{m}", name=f"xt{m}")
                            nfull = uv // 128
                            rem = uv - nfull * 128
                            if nfull:
                                nc.gpsimd.dma_start(
                                    out=xt[:, :nfull, :],
                                    in_=xin[m][s, 0:nfull * 128, :].rearrange(
                                        "(k p) c -> p k c", p=128))
                            if rem:
                                nc.gpsimd.dma_start(
                                    out=xt[:rem, nfull, :],
                                    in_=xin[m][s, nfull * 128:uv, :])
                            for ni, (no, nsz) in enumerate(nch):
                                for Mo, Msz, dst in ((0, 128, x0t),
                                                     (128, 68, x1t)):
                                    pr = pacc.tile([128, 512], F32, tag="pacc")
                                    ks = nzk[m][0 if Mo == 0 else 1]
                                    for idx, ki in enumerate(ks):
                                        ko, ksz = kch[ki]
                                        nc.tensor.matmul(
                                            pr[:Msz, :nsz],
                                            lhsT=rtt[m][:ksz, ki, Mo:Mo + Msz],
                                            rhs=xt[:ksz, ki, no:no + nsz],
                                            start=(idx == 0),
                                            stop=(idx == len(ks) - 1))
                                    _pcopy(out=dst[:Msz, no:no + nsz],
                                           in_=pr[:Msz, :nsz])

                        # zscore along free dim
                        for tl, psz in ((x0t, 128), (x1t, 68)):
                            gs = _chunks(c, 512)
                            st = smalls.tile([128, len(gs), 6], F32, tag="bnst")
                            for gi, (go, gln) in enumerate(gs):
                                nc.vector.bn_stats(out=st[:psz, gi, :],
                                                   in_=tl[:psz, go:go + gln])
                            mv = smalls.tile([128, 2], F32, tag="mv")
                            nc.vector.bn_aggr(out=mv[:psz], in_=st[:psz])
                            sd = smalls.tile([128, 1], F32, tag="sd")
                            nc.scalar.activation(out=sd[:psz], in_=mv[:psz, 1:2],
                                                 func=AF.Sqrt,
                                                 scale=c / (c - 1.0))
                            sc = smalls.tile([128, 1], F32, tag="sc")
                            nc.vector.reciprocal(out=sc[:psz], in_=sd[:psz])
                            bi = smalls.tile([128, 1], F32, tag="bi")
                            nc.vector.tensor_scalar(
                                out=bi[:psz], in0=mv[:psz, 0:1],
                                scalar1=sc[:psz], scalar2=-1.0,
                                op0=ALU.mult, op1=ALU.mult)
                            nc.gpsimd.tensor_scalar(
                                out=tl[:psz], in0=tl[:psz],
                                scalar1=sc[:psz], scalar2=bi[:psz],
                                op0=ALU.mult, op1=ALU.add)

                    # ---- H = sigmoid(W @ Xn) over the sample pair ----
                    hps = hq.tile([16, 2 * WH], F32, tag="hq")
                    for c2 in range(0, cc_n, 2):
                        cn = min(2, cc_n - c2)
                        xn = xnp.tile([128, 2, 2 * WH], F16, tag="xn")
                        for ci in range(cn):
                            ps = pt.tile([128, 512], F16, tag="pt")
                            cc = c2 + ci
                            for si in range(2):
                                off = si * WH
                                nc.tensor.transpose(
                                    ps[:128, off:off + 128],
                                    xnt0[si][:, cc * 128:(cc + 1) * 128],
                                    identh_sb)
                                nc.tensor.transpose(
                                    ps[:128, off + 128:off + 196],
                                    xnt1[si][:68, cc * 128:(cc + 1) * 128],
                                    identh_sb[:68, :68])
                            _pcopy(out=xn[:, ci, :], in_=ps[:, 0:2 * WH])
                        for ci in range(cn):
                            cc = c2 + ci
                            nc.tensor.matmul(
                                hps, lhsT=wtt[m][:, cc, :],
                                rhs=xn[:, ci, :],
                                start=(cc == 0), stop=(cc == cc_n - 1))
                    hsb = hp.tile([16, 2, WH], F16, tag="hsb")
                    nc.scalar.activation(
                        out=hsb.rearrange("p a b -> p (a b)"),
                        in_=hps, func=AF.Sigmoid)

                    htbig = {}
                    for si in range(2):
                        b = 4 * si + m
                        bo = 16 * b
                        htb = htp.tile([128, 2, 128], F16, tag="htbig")
                        nc.gpsimd.tensor_copy(
                            out=htb.rearrange("p a b -> p (a b)"),
                            in_=zer_sb)
                        pht = pt.tile([128, 512], F16, tag="pt")
                        nc.tensor.transpose(pht[:128, 0:Q],
                                            hsb[:Q, si, 0:128],
                                            identh_sb[:Q, :Q])
                        nc.tensor.transpose(pht[:68, Q:2 * Q],
                                            hsb[:Q, si, 128:196],
                                            identh_sb[:Q, :Q])
                        nc.vector.tensor_copy(
                            out=htb[:, 0, bo:bo + Q],
                            in_=pht[:128, 0:Q])
                        nc.vector.tensor_copy(
                            out=htb[:68, 1, bo:bo + Q],
                            in_=pht[:68, Q:2 * Q])
                        htbig[si] = htb
                        # G block accumulation into SBUF accumulator
                        gp = nsp.tile([128, 128], F32, tag="ns")
                        for wi, wsz in ((0, 128), (1, 68)):
                            nc.tensor.matmul(gp,
                                             lhsT=htb[:wsz, wi, :],
                                             rhs=htb[:wsz, wi, :],
                                             start=(wi == 0), stop=(wi == 1))
                        nc.vector.tensor_tensor(out=gsup[spr], in0=gsup[spr],
                                                in1=gp, op=ALU.add)
                    # C^T for both blocks of the pair
                    for ni, (no, nsz) in enumerate(nch):
                        cps = pacc.tile([128, 512], F32, tag="pacc")
                        idx = 0
                        for si in range(2):
                            for wi, wsz, xs in ((0, 128, xnt0[si]),
                                                (1, 68, xnt1[si])):
                                nc.tensor.matmul(
                                    cps[:, :nsz],
                                    lhsT=htbig[si][:wsz, wi, :],
                                    rhs=xs[:wsz, no:no + nsz],
                                    start=(idx == 0), stop=(idx == 3))
                                idx += 1
                        _pcopy(out=ct_all[spr][:, OFFS[m] + no:OFFS[m] + no +
                                               nsz],
                               in_=cps[:, :nsz])

                # ---- phase 2 for this super: Newton-Schulz ----
                # fp16 iterations (PE 1 cyc/row) + one fp32 polish iteration;
                # the polish contracts fp16 iteration error quadratically.
                g = gsup[spr]
                gh = g
                sq = newt.tile([128, 128], F32, tag="sq")
                nc.vector.tensor_mul(sq, g, g)
                rs = newt.tile([128, 1], F32, tag="rs")
                nc.vector.tensor_reduce(out=rs, in_=sq,
                                        axis=mybir.AxisListType.X, op=ALU.add)
                bps = nsp.tile([128, 128], F32, tag="ns")
                nc.tensor.matmul(bps[:128, 0:1], lhsT=p16_sb, rhs=rs,
                                 start=True, stop=True)
                bf = newt.tile([128, 1], F32, tag="bf")
                nc.scalar.activation(out=bf, in_=bps[:128, 0:1], func=AF.Sqrt)
                al = newt.tile([128, 1], F32, tag="al")
                nc.vector.reciprocal(out=al, in_=bf)
                xh = newt.tile([128, 128], F32, tag="xh")
                nc.vector.tensor_scalar_mul(xh, ident_sb, al)
                xps = None
                for it in range(NEWTON_ITERS):
                    yps = nsp.tile([128, 128], F32, tag="ns")
                    nc.tensor.matmul(yps, lhsT=gh, rhs=xh,
                                     start=True, stop=True)
                    zh = newt.tile([128, 128], F32, tag="zh")
                    nc.vector.scalar_tensor_tensor(
                        out=zh, in0=ident_sb, scalar=2.0,
                        in1=yps, op0=ALU.mult, op1=ALU.subtract)
                    xps = nsp.tile([128, 128], F32, tag="ns")
                    nc.tensor.matmul(xps, lhsT=xh, rhs=zh,
                                     start=True, stop=True)
                    if it < NEWTON_ITERS - 1:
                        xh = newt.tile([128, 128], F32, tag="xh")
                        nc.scalar.copy(out=xh, in_=xps)
                x32 = newt.tile([128, 128], F32, tag="x32")
                nc.scalar.copy(out=x32, in_=xps)
                yp2 = nsp.tile([128, 128], F32, tag="ns")
                nc.tensor.matmul(yp2, lhsT=g, rhs=x32, start=True, stop=True)
                z32 = newt.tile([128, 128], F32, tag="z32")
                nc.vector.scalar_tensor_tensor(
                    out=z32, in0=ident_sb, scalar=2.0,
                    in1=yp2, op0=ALU.mult, op1=ALU.subtract)
                xp2 = nsp.tile([128, 128], F32, tag="ns")
                nc.tensor.matmul(xp2, lhsT=x32, rhs=z32, start=True, stop=True)
                xf = newt.tile([128, 128], F32, tag="xf")
                nc.scalar.copy(out=xf, in_=xp2)
                mps = nsp.tile([128, 128], F32, tag="ns")
                nc.tensor.matmul(mps, lhsT=xf, rhs=xf,
                                 start=True, stop=True)
                m2t = sup.tile([128, 128], F16, tag=f"m2_{spr}",
                               name=f"m2_{spr}")
                nc.vector.tensor_copy(out=m2t, in_=mps)
                m2_sb[spr] = m2t
                rps = nsp.tile([128, 128], F32, tag="ns")
                nc.tensor.matmul(rps[:128, 0:1], lhsT=xf, rhs=ones_sb,
                                 start=True, stop=True)
                rt_ = sup.tile([128, 1], F32, tag=f"r_{spr}", name=f"r_{spr}")
                nc.vector.tensor_copy(out=rt_, in_=rps[:128, 0:1])
                r_sb[spr] = rt_

            # r4[m][spr] = mask8[m, spr] * r_super(spr)
            r4 = []
            for m in range(4):
                t = smalls.tile([128, 2, 4], F16, tag="r4", name=f"r4_{m}")
                nc.vector.tensor_scalar_mul(t[:, 0, :],
                                            mask8_sb[:, m, 0, :], r_sb[0])
                nc.vector.tensor_scalar_mul(t[:, 1, :],
                                            mask8_sb[:, m, 1, :], r_sb[1])
                r4.append(t)

            # ================= phase 3: variance readout =================
            for m, (c, sp) in enumerate(MEMBERS):
                for ni, (no, nsz) in enumerate(_chunks(c, 512)):
                    g0 = OFFS[m] + no
                    psbs = []
                    for spr in range(2):
                        dfp = pacc.tile([128, 512], F32, tag="pacc")
                        nc.tensor.matmul(
                            dfp[:, :nsz], lhsT=m2_sb[spr],
                            rhs=ct_all[spr][:, g0:g0 + nsz],
                            start=True, stop=True)
                        psb = psbp.tile([128, 512], F16, tag="psb")
                        nc.vector.tensor_mul(psb[:, :nsz],
                                             ct_all[spr][:, g0:g0 + nsz],
                                             dfp[:, :nsz])
                        psbs.append(psb)
                    qps = hq.tile([4, 512], F32, tag="hq")
                    tps = hq.tile([4, 512], F32, tag="hq")
                    for spr in range(2):
                        nc.tensor.matmul(
                            qps[:, :nsz],
                            lhsT=oq8_sb[:, m, spr, :],
                            rhs=psbs[spr][:, :nsz],
                            start=(spr == 0), stop=(spr == 1))
                        nc.tensor.matmul(
                            tps[:, :nsz], lhsT=r4[m][:, spr, :],
                            rhs=ct_all[spr][:, g0:g0 + nsz],
                            start=(spr == 0), stop=(spr == 1))
                    tsq = tsqp.tile([S, 512], F32, tag="tsq")
                    nc.scalar.square(out=tsq[:, :nsz], in_=tps[:S, :nsz])
                    ot = outp.tile([S, 512], F32, tag="out")
                    nc.vector.scalar_tensor_tensor(
                        out=ot[:, :nsz],
                        in0=tsq[:, :nsz], scalar=-1.0 / ((Q - 1) * Q),
                        in1=qps[:S, :nsz], op0=ALU.mult, op1=ALU.add)
                    nc.sync.dma_start(out=outd[:, g0:g0 + nsz],
                                      in_=ot[:, :nsz])

    nc.finalize()
    return nc


_CACHE = {}


def _in_maps(inputs):
    rts, ident_np, p16_np, mask8_np = _consts()
    xs = [np.asarray(inputs[f"x{m}"]) for m in range(4)]
    ws = [np.asarray(inputs[f"W{m}"]) for m in range(4)]
    in_maps = []
    for i in range(NCORES):
        im = {"ident": ident_np, "p16": p16_np, "mask8": mask8_np}
        for m, (c, sp) in enumerate(MEMBERS):
            xi = xs[m][S * i:S * (i + 1)].reshape(S, c, sp * sp)
            im[f"x{m}"] = np.ascontiguousarray(
                xi.transpose(0, 2, 1), np.float16)      # [S, uv, c] fp16
            im[f"wt{m}"] = np.ascontiguousarray(ws[m].T, np.float32)
            if m in rts:
                im[f"rt{m}"] = rts[m]
        in_maps.append(im)
    return in_maps


def kernel(x0, x1, x2, x3, W0, W1, W2, W3):
    if "nc" not in _CACHE:
        _CACHE["nc"] = _build_program()
    nc = _CACHE["nc"]
    in_maps = _in_maps(dict(x0=x0, x1=x1, x2=x2, x3=x3,
                            W0=W0, W1=W1, W2=W2, W3=W3))
    res = run_bass_kernel_spmd(nc, in_maps, list(range(NCORES)))
    return np.concatenate([r["out"] for r in res.results], axis=0)


# revision 19
# speedup vs baseline: 1.3051x; 1.0680x over previous
"""Trainium2 Bass kernel for nn_ELM_AE_FatSpectral_Ensemble.

Data-parallel over batch: 4 samples/core on 8 cores. Per (sample, member):
  XrT[w,c] = RfullT.T @ xT   (bilinear+antialias resize via PE matmul)
  XnT      = zscore(XrT) along c (bn_stats + per-partition scale on free dim)
  H        = sigmoid(W @ Xn);  G = H H^T (16x16)
  G^-1 via Newton-Schulz on block-diagonal [128,128] supermatrices
  var(B), B = C G^-1, C = Xn H^T:
     quad_i = C_i G^-2 C_i^T, t_i = C_i (G^-1 1)
     out_i = quad_i/(Q-1) - t_i^2/(Q(Q-1))
Blocks of the supermatrix: b = 4*si + m (si = sample within pair), one
supermatrix per sample pair (spr), built super-outer so Newton-Schulz of
super 0 overlaps phase 1 of super 1.

V3: inputs are transposed on the host ([S, uv, c] layout) so the PE never
transposes raw x; DMA-cast fp32->fp16 lands directly in the resize rhs
layout at line rate. Heavy path all fp16 (PE 1 cycle/row + FWL); PSUM
accumulation fp32; Newton-Schulz fp32.
"""

import numpy as np

import concourse.bacc as bacc
import concourse.tile as tile
from concourse import mybir
from concourse.bass_utils import run_bass_kernel_spmd

F32 = mybir.dt.float32
F16 = mybir.dt.float16
AF = mybir.ActivationFunctionType
ALU = mybir.AluOpType

S = 4
NCORES = 8
SP = 14
WH = SP * SP
Q = 16
MEMBERS = [(256, 56), (512, 28), (1024, 14), (2048, 7)]
OFFS = [0, 256, 768, 1792]
DTOT = 3840
NEWTON_ITERS = 10
MORDER = [3, 2, 1, 0]     # small members first: shortens the startup DMA stall


def _weight_mat(n_in, n_out):
    scale = n_out / n_in
    kernel_scale = max(1.0, 1.0 / scale)
    sample_f = (np.arange(n_out) + 0.5) / scale - 0.5
    x = np.abs(sample_f[:, None] - np.arange(n_in)[None, :]) / kernel_scale
    w = np.maximum(0.0, 1.0 - x)
    total = w.sum(axis=1, keepdims=True)
    return (w / np.where(total > 0, total, 1)).astype(np.float32)


def _consts():
    rts = {}
    for m, (c, sp) in enumerate(MEMBERS):
        if sp == SP:
            continue
        R = _weight_mat(sp, SP)
        rt = np.kron(R, R).T.astype(np.float32)   # [uv, 196]
        pad = (-rt.shape[0]) % 128
        if pad:
            rt = np.concatenate([rt, np.zeros((pad, WH), np.float32)], 0)
        rts[m] = np.ascontiguousarray(rt)
    ident = np.eye(128, dtype=np.float32)
    p16 = np.kron(np.eye(8, dtype=np.float32), np.ones((16, 16), np.float32))
    p16 = p16.astype(np.float32)
    # mask8[m, spr][p, col]: per-super sample masks
    mask8 = np.zeros((4, 2, 128, 4), np.float32)
    for m in range(4):
        for spr in range(2):
            for si in range(2):
                b = 4 * si + m
                mask8[m, spr, 16 * b:16 * b + 16, 2 * spr + si] = 1.0
    mask8 = np.ascontiguousarray(mask8.transpose(2, 0, 1, 3))  # [128, 4, 2, 4]
    return rts, ident, p16, mask8


def _chunks(n, sz=128):
    return [(i, min(sz, n - i)) for i in range(0, n, sz)]


def _build_program(debug=False):
    rts, ident_np, p16_np, mask8_np = _consts()

    nc = bacc.Bacc()
    xin, wt, rtd = {}, {}, {}
    for m, (c, sp) in enumerate(MEMBERS):
        # host-transposed fp16 input: [S, uv, c]
        xin[m] = nc.dram_tensor(f"x{m}", [S, sp * sp, c], F16,
                                kind="ExternalInput")
        wt[m] = nc.dram_tensor(f"wt{m}", [c, Q], F32, kind="ExternalInput")
        if m in rts:
            rtd[m] = nc.dram_tensor(f"rt{m}", list(rts[m].shape), F32,
                                    kind="ExternalInput")
    identd = nc.dram_tensor("ident", [128, 128], F32, kind="ExternalInput")
    p16d = nc.dram_tensor("p16", [128, 128], F32, kind="ExternalInput")
    mask8d = nc.dram_tensor("mask8", [128, 4, 2, 4], F32, kind="ExternalInput")
    outd = nc.dram_tensor("out", [S, DTOT], F32, kind="ExternalOutput")

    nzk = {}
    for m in rts:
        uv = MEMBERS[m][1] ** 2
        nzk[m] = {}
        for Mi, (Mo, Msz) in enumerate([(0, 128), (128, 68)]):
            nzk[m][Mi] = [ki for ki, (ko, ksz) in enumerate(_chunks(uv))
                          if np.any(rts[m][ko:ko + ksz, Mo:Mo + Msz] != 0)]

    from contextlib import ExitStack
    _ceng = [0]

    def _pcopy(out, in_):
        _ceng[0] ^= 1
        (nc.scalar.copy if _ceng[0] else nc.vector.tensor_copy)(out=out, in_=in_)

    with tile.TileContext(nc) as tc, ExitStack() as _es:
        _p = lambda **kw: _es.enter_context(tc.tile_pool(**kw))
        consts = _p(name="consts", bufs=1)
        xtp = _p(name="xtp", bufs=3)
        xntp = _p(name="xntp", bufs=2)
        xnp = _p(name="xnp", bufs=3)
        hp = _p(name="hp", bufs=2)
        htp = _p(name="htp", bufs=2)
        sup = _p(name="sup", bufs=1)
        newt = _p(name="newt", bufs=2)
        smalls = _p(name="smalls", bufs=4)
        psbp = _p(name="psbp", bufs=2)
        tsqp = _p(name="tsqp", bufs=2)
        outp = _p(name="outp", bufs=2)
        # PSUM budget (16KB/partition): pt 2x1KB + big512 3x2KB + ns 2x0.5KB
        # + hq 3x2KB = 15KB
        pt = _p(name="pt", bufs=2, space="PSUM")
        pacc = _p(name="pacc", bufs=3, space="PSUM")
        nsp = _p(name="nsp", bufs=1, space="PSUM")
        hq = _p(name="hq", bufs=2, space="PSUM")
        if True:
            # ---------- constants ----------
            ident_sb = consts.tile([128, 128], F32, tag="ident")
            nc.gpsimd.dma_start(out=ident_sb, in_=identd[:, :])
            identh_sb = consts.tile([128, 128], F16, tag="identh")
            nc.gpsimd.dma_start(out=identh_sb, in_=identd[:, :])
            p16_sb = consts.tile([128, 128], F32, tag="p16")
            nc.gpsimd.dma_start(out=p16_sb, in_=p16d[:, :])
            mask8_sb = consts.tile([128, 4, 2, 4], F32, tag="mask8")
            nc.gpsimd.dma_start(out=mask8_sb, in_=mask8d[:, :, :, :])
            oq8_sb = consts.tile([128, 4, 2, 4], F16, tag="oq8")
            nc.vector.tensor_scalar_mul(
                oq8_sb.rearrange("p a b c -> p (a b c)"),
                mask8_sb.rearrange("p a b c -> p (a b c)"), 1.0 / (Q - 1))
            ones_sb = consts.tile([128, 1], F32, tag="ones")
            nc.vector.memset(ones_sb, 1.0)
            zer_sb = consts.tile([128, 256], F16, tag="zer")
            nc.vector.memset(zer_sb, 0.0)

            rtt, wtt = {}, {}
            for m, (c, sp) in enumerate(MEMBERS):
                cc_n = c // 128
                wtt[m] = consts.tile([128, cc_n, Q], F16, tag=f"wt{m}",
                                     name=f"wt{m}")
                nc.gpsimd.dma_start(
                    out=wtt[m],
                    in_=wt[m][:, :].rearrange("(k p) q -> p k q", p=128))
                if m in rts:
                    nk = rts[m].shape[0] // 128
                    rtt[m] = consts.tile([128, nk, WH], F16, tag=f"rt{m}",
                                         name=f"rt{m}")
                    nc.gpsimd.dma_start(
                        out=rtt[m],
                        in_=rtd[m][:, :].rearrange("(k p) w -> p k w", p=128))

            ct_all, gsup = [], []
            for spr in range(2):
                t = sup.tile([128, DTOT], F16, tag=f"ct{spr}", name=f"ct{spr}")
                ct_all.append(t)
                g = sup.tile([128, 128], F32, tag=f"gsup{spr}",
                             name=f"gsup{spr}")
                nc.vector.memset(g, 0.0)
                gsup.append(g)

            m2_sb = [None, None]
            r_sb = [None, None]

            # ============ phase 1+2, super-outer =============
            for spr in range(2):
                for m in MORDER:
                    c, sp = MEMBERS[m]
                    uv = sp * sp
                    cc_n = c // 128
                    kch = _chunks(uv)
                    nch = _chunks(c, 512)

                    xnt0, xnt1 = {}, {}
                    for si in range(2):
                        s = 2 * spr + si
                        x0t = xntp.tile([128, c], F16, tag=f"xnt0_{si}",
                                        name=f"xnt0_{si}")
                        x1t = xntp.tile([68, c], F16, tag=f"xnt1_{si}",
                                        name=f"xnt1_{si}")
                        xnt0[si], xnt1[si] = x0t, x1t

                        if m == 2:
                            # identity resize: XrT comes straight from DRAM
                            nc.gpsimd.dma_start(out=x0t,
                                                in_=xin[m][s, 0:128, :])
                            nc.gpsimd.dma_start(out=x1t[:68, :],
                                                in_=xin[m][s, 128:196, :])
                        else:
                            # xt = x^T [uv(part), c] fp16, direct DMA-cast
                            xt = xtp.tile([128, len(kch), c], F16,
                                          tag=f"xt{m}", name=f"xt{m}")
                            nfull = uv // 128
                            rem = uv - nfull * 128
                            if nfull:
                                nc.gpsimd.dma_start(
                                    out=xt[:, :nfull, :],
                                    in_=xin[m][s, 0:nfull * 128, :].rearrange(
                                        "(k p) c -> p k c", p=128))
                            if rem:
                                nc.gpsimd.dma_start(
                                    out=xt[:rem, nfull, :],
                                    in_=xin[m][s, nfull * 128:uv, :])
                            for ni, (no, nsz) in enumerate(nch):
                                for Mo, Msz, dst in ((0, 128, x0t),
                                                     (128, 68, x1t)):
                                    pr = pacc.tile([128, 512], F32, tag="pacc")
                                    ks = nzk[m][0 if Mo == 0 else 1]
                                    for idx, ki in enumerate(ks):
                                        ko, ksz = kch[ki]
                                        nc.tensor.matmul(
                                            pr[:Msz, :nsz],
                                            lhsT=rtt[m][:ksz, ki, Mo:Mo + Msz],
                                            rhs=xt[:ksz, ki, no:no + nsz],
                                            start=(idx == 0),
                                            stop=(idx == len(ks) - 1))
                                    _pcopy(out=dst[:Msz, no:no + nsz],
                                           in_=pr[:Msz, :nsz])

                        # zscore along free dim
                        for tl, psz in ((x0t, 128), (x1t, 68)):
                            gs = _chunks(c, 512)
                            st = smalls.tile([128, len(gs), 6], F32, tag="bnst")
                            for gi, (go, gln) in enumerate(gs):
                                nc.vector.bn_stats(out=st[:psz, gi, :],
                                                   in_=tl[:psz, go:go + gln])
                            mv = smalls.tile([128, 2], F32, tag="mv")
                            nc.vector.bn_aggr(out=mv[:psz], in_=st[:psz])
                            sd = smalls.tile([128, 1], F32, tag="sd")
                            nc.scalar.activation(out=sd[:psz], in_=mv[:psz, 1:2],
                                                 func=AF.Sqrt,
                                                 scale=c / (c - 1.0))
                            sc = smalls.tile([128, 1], F32, tag="sc")
                            nc.vector.reciprocal(out=sc[:psz], in_=sd[:psz])
                            bi = smalls.tile([128, 1], F32, tag="bi")
                            nc.vector.tensor_scalar(
                                out=bi[:psz], in0=mv[:psz, 0:1],
                                scalar1=sc[:psz], scalar2=-1.0,
                                op0=ALU.mult, op1=ALU.mult)
                            nc.gpsimd.tensor_scalar(
                                out=tl[:psz], in0=tl[:psz],
                                scalar1=sc[:psz], scalar2=bi[:psz],
                                op0=ALU.mult, op1=ALU.add)

                    # ---- H = sigmoid(W @ Xn) over the sample pair ----
                    hps = hq.tile([16, 2 * WH], F32, tag="hq")
                    for c2 in range(0, cc_n, 2):
                        cn = min(2, cc_n - c2)
                        xn = xnp.tile([128, 2, 2 * WH], F16, tag="xn")
                        for ci in range(cn):
                            ps = pt.tile([128, 512], F16, tag="pt")
                            cc = c2 + ci
                            for si in range(2):
                                off = si * WH
                                nc.tensor.transpose(
                                    ps[:128, off:off + 128],
                                    xnt0[si][:, cc * 128:(cc + 1) * 128],
                                    identh_sb)
                                nc.tensor.transpose(
                                    ps[:128, off + 128:off + 196],
                                    xnt1[si][:68, cc * 128:(cc + 1) * 128],
                                    identh_sb[:68, :68])
                            _pcopy(out=xn[:, ci, :], in_=ps[:, 0:2 * WH])
                        for ci in range(cn):
                            cc = c2 + ci
                            nc.tensor.matmul(
                                hps, lhsT=wtt[m][:, cc, :],
                                rhs=xn[:, ci, :],
                                start=(cc == 0), stop=(cc == cc_n - 1))
                    hsb = hp.tile([16, 2, WH], F16, tag="hsb")
                    nc.scalar.activation(
                        out=hsb.rearrange("p a b -> p (a b)"),
                        in_=hps, func=AF.Sigmoid)

                    htbig = {}
                    for si in range(2):
                        b = 4 * si + m
                        bo = 16 * b
                        htb = htp.tile([128, 2, 128], F16, tag="htbig")
                        nc.gpsimd.tensor_copy(
                            out=htb.rearrange("p a b -> p (a b)"),
                            in_=zer_sb)
                        pht = pt.tile([128, 512], F16, tag="pt")
                        nc.tensor.transpose(pht[:128, 0:Q],
                                            hsb[:Q, si, 0:128],
                                            identh_sb[:Q, :Q])
                        nc.tensor.transpose(pht[:68, Q:2 * Q],
                                            hsb[:Q, si, 128:196],
                                            identh_sb[:Q, :Q])
                        nc.vector.tensor_copy(
                            out=htb[:, 0, bo:bo + Q],
                            in_=pht[:128, 0:Q])
                        nc.vector.tensor_copy(
                            out=htb[:68, 1, bo:bo + Q],
                            in_=pht[:68, Q:2 * Q])
                        htbig[si] = htb
                        # G block accumulation into SBUF accumulator
                        gp = nsp.tile([128, 128], F32, tag="ns")
                        for wi, wsz in ((0, 128), (1, 68)):
                            nc.tensor.matmul(gp,
                                             lhsT=htb[:wsz, wi, :],
                                             rhs=htb[:wsz, wi, :],
                                             start=(wi == 0), stop=(wi == 1))
                        nc.vector.tensor_tensor(out=gsup[spr], in0=gsup[spr],
                                                in1=gp, op=ALU.add)
                    # C^T for both blocks of the pair
                    for ni, (no, nsz) in enumerate(nch):
                        cps = pacc.tile([128, 512], F32, tag="pacc")
                        idx = 0
                        for si in range(2):
                            for wi, wsz, xs in ((0, 128, xnt0[si]),
                                                (1, 68, xnt1[si])):
                                nc.tensor.matmul(
                                    cps[:, :nsz],
                                    lhsT=htbig[si][:wsz, wi, :],
                                    rhs=xs[:wsz, no:no + nsz],
                                    start=(idx == 0), stop=(idx == 3))
                                idx += 1
                        _pcopy(out=ct_all[spr][:, OFFS[m] + no:OFFS[m] + no +
                                               nsz],
                               in_=cps[:, :nsz])

                # ---- phase 2 for this super: Newton-Schulz ----
                # fp16 iterations (PE 1 cyc/row) + one fp32 polish iteration;
                # the polish contracts fp16 iteration error quadratically.
                g = gsup[spr]
                gh = newt.tile([128, 128], F16, tag="gh")
                nc.vector.tensor_copy(out=gh, in_=g)
                sq = newt.tile([128, 128], F32, tag="sq")
                nc.vector.tensor_mul(sq, g, g)
                rs = newt.tile([128, 1], F32, tag="rs")
                nc.vector.tensor_reduce(out=rs, in_=sq,
                                        axis=mybir.AxisListType.X, op=ALU.add)
                bps = nsp.tile([128, 128], F32, tag="ns")
                nc.tensor.matmul(bps[:128, 0:1], lhsT=p16_sb, rhs=rs,
                                 start=True, stop=True)
                bf = newt.tile([128, 1], F32, tag="bf")
                nc.scalar.activation(out=bf, in_=bps[:128, 0:1], func=AF.Sqrt)
                al = newt.tile([128, 1], F32, tag="al")
                nc.vector.reciprocal(out=al, in_=bf)
                xh = newt.tile([128, 128], F16, tag="xh")
                nc.vector.tensor_scalar_mul(xh, ident_sb, al)
                xps = None
                for it in range(NEWTON_ITERS):
                    yps = nsp.tile([128, 128], F32, tag="ns")
                    nc.tensor.matmul(yps, lhsT=gh, rhs=xh,
                                     start=True, stop=True)
                    zh = newt.tile([128, 128], F16, tag="zh")
                    nc.vector.scalar_tensor_tensor(
                        out=zh, in0=ident_sb, scalar=2.0,
                        in1=yps, op0=ALU.mult, op1=ALU.subtract)
                    xps = nsp.tile([128, 128], F32, tag="ns")
                    nc.tensor.matmul(xps, lhsT=xh, rhs=zh,
                                     start=True, stop=True)
                    if it < NEWTON_ITERS - 1:
                        xh = newt.tile([128, 128], F16, tag="xh")
                        nc.scalar.copy(out=xh, in_=xps)
                x32 = newt.tile([128, 128], F32, tag="x32")
                nc.scalar.copy(out=x32, in_=xps)
                # symmetrize: fp16 iterations accumulate an antisymmetric
                # error component the polish cannot contract
                xtps = nsp.tile([128, 128], F32, tag="ns")
                nc.tensor.transpose(xtps, x32, ident_sb)
                xsum = newt.tile([128, 128], F32, tag="xsum")
                nc.vector.tensor_tensor(out=xsum, in0=x32, in1=xtps,
                                        op=ALU.add)
                x32 = newt.tile([128, 128], F32, tag="x32")
                nc.scalar.mul(out=x32, in_=xsum, mul=0.5)
                for _pol in range(2):
                    yp2 = nsp.tile([128, 128], F32, tag="ns")
                    nc.tensor.matmul(yp2, lhsT=g, rhs=x32, start=True,
                                     stop=True)
                    z32 = newt.tile([128, 128], F32, tag="z32")
                    nc.vector.scalar_tensor_tensor(
                        out=z32, in0=ident_sb, scalar=2.0,
                        in1=yp2, op0=ALU.mult, op1=ALU.subtract)
                    xp2 = nsp.tile([128, 128], F32, tag="ns")
                    nc.tensor.matmul(xp2, lhsT=x32, rhs=z32, start=True,
                                     stop=True)
                    x32 = newt.tile([128, 128], F32, tag="x32")
                    nc.scalar.copy(out=x32, in_=xp2)
                xf = x32
                mps = nsp.tile([128, 128], F32, tag="ns")
                nc.tensor.matmul(mps, lhsT=xf, rhs=xf,
                                 start=True, stop=True)
                m2t = sup.tile([128, 128], F16, tag=f"m2_{spr}",
                               name=f"m2_{spr}")
                nc.vector.tensor_copy(out=m2t, in_=mps)
                m2_sb[spr] = m2t
                rps = nsp.tile([128, 128], F32, tag="ns")
                nc.tensor.matmul(rps[:128, 0:1], lhsT=xf, rhs=ones_sb,
                                 start=True, stop=True)
                rt_ = sup.tile([128, 1], F32, tag=f"r_{spr}", name=f"r_{spr}")
                nc.vector.tensor_copy(out=rt_, in_=rps[:128, 0:1])
                r_sb[spr] = rt_

            # r4[m][spr] = mask8[m, spr] * r_super(spr)
            r4 = []
            for m in range(4):
                t = smalls.tile([128, 2, 4], F16, tag="r4", name=f"r4_{m}")
                nc.vector.tensor_scalar_mul(t[:, 0, :],
                                            mask8_sb[:, m, 0, :], r_sb[0])
                nc.vector.tensor_scalar_mul(t[:, 1, :],
                                            mask8_sb[:, m, 1, :], r_sb[1])
                r4.append(t)

            # ================= phase 3: variance readout =================
            for m, (c, sp) in enumerate(MEMBERS):
                for ni, (no, nsz) in enumerate(_chunks(c, 512)):
                    g0 = OFFS[m] + no
                    psbs = []
                    for spr in range(2):
                        dfp = pacc.tile([128, 512], F32, tag="pacc")
                        nc.tensor.matmul(
                            dfp[:, :nsz], lhsT=m2_sb[spr],
                            rhs=ct_all[spr][:, g0:g0 + nsz],
                            start=True, stop=True)
                        psb = psbp.tile([128, 512], F16, tag="psb")
                        nc.vector.tensor_mul(psb[:, :nsz],
                                             ct_all[spr][:, g0:g0 + nsz],
                                             dfp[:, :nsz])
                        psbs.append(psb)
                    qps = hq.tile([4, 512], F32, tag="hq")
                    tps = hq.tile([4, 512], F32, tag="hq")
                    for spr in range(2):
                        nc.tensor.matmul(
                            qps[:, :nsz],
                            lhsT=oq8_sb[:, m, spr, :],
                            rhs=psbs[spr][:, :nsz],
                            start=(spr == 0), stop=(spr == 1))
                        nc.tensor.matmul(
                            tps[:, :nsz], lhsT=r4[m][:, spr, :],
                            rhs=ct_all[spr][:, g0:g0 + nsz],
                            start=(spr == 0), stop=(spr == 1))
                    tsq = tsqp.tile([S, 512], F32, tag="tsq")
                    nc.scalar.square(out=tsq[:, :nsz], in_=tps[:S, :nsz])
                    ot = outp.tile([S, 512], F32, tag="out")
                    nc.vector.scalar_tensor_tensor(
                        out=ot[:, :nsz],
                        in0=tsq[:, :nsz], scalar=-1.0 / ((Q - 1) * Q),
                        in1=qps[:S, :nsz], op0=ALU.mult, op1=ALU.add)
                    nc.sync.dma_start(out=outd[:, g0:g0 + nsz],
                                      in_=ot[:, :nsz])

    nc.finalize()
    return nc


_CACHE = {}


def _in_maps(inputs):
    rts, ident_np, p16_np, mask8_np = _consts()
    xs = [np.asarray(inputs[f"x{m}"]) for m in range(4)]
    ws = [np.asarray(inputs[f"W{m}"]) for m in range(4)]
    in_maps = []
    for i in range(NCORES):
        im = {"ident": ident_np, "p16": p16_np, "mask8": mask8_np}
        for m, (c, sp) in enumerate(MEMBERS):
            xi = xs[m][S * i:S * (i + 1)].reshape(S, c, sp * sp)
            im[f"x{m}"] = np.ascontiguousarray(
                xi.transpose(0, 2, 1), np.float16)      # [S, uv, c] fp16
            im[f"wt{m}"] = np.ascontiguousarray(ws[m].T, np.float32)
            if m in rts:
                im[f"rt{m}"] = rts[m]
        in_maps.append(im)
    return in_maps


def kernel(x0, x1, x2, x3, W0, W1, W2, W3):
    if "nc" not in _CACHE:
        _CACHE["nc"] = _build_program()
    nc = _CACHE["nc"]
    in_maps = _in_maps(dict(x0=x0, x1=x1, x2=x2, x3=x3,
                            W0=W0, W1=W1, W2=W2, W3=W3))
    res = run_bass_kernel_spmd(nc, in_maps, list(range(NCORES)))
    return np.concatenate([r["out"] for r in res.results], axis=0)


# revision 20
# speedup vs baseline: 1.3368x; 1.0243x over previous
"""Trainium2 Bass kernel for nn_ELM_AE_FatSpectral_Ensemble.

Data-parallel over batch: 4 samples/core on 8 cores. Per (sample, member):
  XrT[w,c] = RfullT.T @ xT   (bilinear+antialias resize via PE matmul)
  XnT      = zscore(XrT) along c (bn_stats + per-partition scale on free dim)
  H        = sigmoid(W @ Xn);  G = H H^T (16x16)
  G^-1 via Newton-Schulz on block-diagonal [128,128] supermatrices
  var(B), B = C G^-1, C = Xn H^T:
     quad_i = C_i G^-2 C_i^T, t_i = C_i (G^-1 1)
     out_i = quad_i/(Q-1) - t_i^2/(Q(Q-1))
Blocks of the supermatrix: b = 4*si + m (si = sample within pair), one
supermatrix per sample pair (spr), built super-outer so Newton-Schulz of
super 0 overlaps phase 1 of super 1.

V3: inputs are transposed on the host ([S, uv, c] layout) so the PE never
transposes raw x; DMA-cast fp32->fp16 lands directly in the resize rhs
layout at line rate. Heavy path all fp16 (PE 1 cycle/row + FWL); PSUM
accumulation fp32; Newton-Schulz fp32.
"""

import numpy as np

import concourse.bacc as bacc
import concourse.tile as tile
from concourse import mybir
from concourse.bass_utils import run_bass_kernel_spmd

F32 = mybir.dt.float32
F16 = mybir.dt.float16
AF = mybir.ActivationFunctionType
ALU = mybir.AluOpType

S = 4
NCORES = 8
SP = 14
WH = SP * SP
Q = 16
MEMBERS = [(256, 56), (512, 28), (1024, 14), (2048, 7)]
OFFS = [0, 256, 768, 1792]
DTOT = 3840
NEWTON_ITERS = 9
MORDER = [3, 2, 1, 0]     # small members first: shortens the startup DMA stall


def _weight_mat(n_in, n_out):
    scale = n_out / n_in
    kernel_scale = max(1.0, 1.0 / scale)
    sample_f = (np.arange(n_out) + 0.5) / scale - 0.5
    x = np.abs(sample_f[:, None] - np.arange(n_in)[None, :]) / kernel_scale
    w = np.maximum(0.0, 1.0 - x)
    total = w.sum(axis=1, keepdims=True)
    return (w / np.where(total > 0, total, 1)).astype(np.float32)


def _consts():
    rts = {}
    for m, (c, sp) in enumerate(MEMBERS):
        if sp == SP:
            continue
        R = _weight_mat(sp, SP)
        rt = np.kron(R, R).T.astype(np.float32)   # [uv, 196]
        pad = (-rt.shape[0]) % 128
        if pad:
            rt = np.concatenate([rt, np.zeros((pad, WH), np.float32)], 0)
        rts[m] = np.ascontiguousarray(rt)
    ident = np.eye(128, dtype=np.float32)
    p16 = np.kron(np.eye(8, dtype=np.float32), np.ones((16, 16), np.float32))
    p16 = p16.astype(np.float32)
    # mask8[m, spr][p, col]: per-super sample masks
    mask8 = np.zeros((4, 2, 128, 4), np.float32)
    for m in range(4):
        for spr in range(2):
            for si in range(2):
                b = 4 * si + m
                mask8[m, spr, 16 * b:16 * b + 16, 2 * spr + si] = 1.0
    mask8 = np.ascontiguousarray(mask8.transpose(2, 0, 1, 3))  # [128, 4, 2, 4]
    return rts, ident, p16, mask8


def _chunks(n, sz=128):
    return [(i, min(sz, n - i)) for i in range(0, n, sz)]


def _build_program(debug=False):
    rts, ident_np, p16_np, mask8_np = _consts()

    nc = bacc.Bacc()
    xin, wt, rtd = {}, {}, {}
    for m, (c, sp) in enumerate(MEMBERS):
        # host-transposed fp16 input: [S, uv, c]
        xin[m] = nc.dram_tensor(f"x{m}", [S, sp * sp, c], F16,
                                kind="ExternalInput")
        wt[m] = nc.dram_tensor(f"wt{m}", [c, Q], F32, kind="ExternalInput")
        if m in rts:
            rtd[m] = nc.dram_tensor(f"rt{m}", list(rts[m].shape), F32,
                                    kind="ExternalInput")
    identd = nc.dram_tensor("ident", [128, 128], F32, kind="ExternalInput")
    p16d = nc.dram_tensor("p16", [128, 128], F32, kind="ExternalInput")
    mask8d = nc.dram_tensor("mask8", [128, 4, 2, 4], F32, kind="ExternalInput")
    outd = nc.dram_tensor("out", [S, DTOT], F32, kind="ExternalOutput")

    nzk = {}
    for m in rts:
        uv = MEMBERS[m][1] ** 2
        nzk[m] = {}
        for Mi, (Mo, Msz) in enumerate([(0, 128), (128, 68)]):
            nzk[m][Mi] = [ki for ki, (ko, ksz) in enumerate(_chunks(uv))
                          if np.any(rts[m][ko:ko + ksz, Mo:Mo + Msz] != 0)]

    from contextlib import ExitStack
    _ceng = [0]

    def _pcopy(out, in_):
        _ceng[0] ^= 1
        (nc.scalar.copy if _ceng[0] else nc.vector.tensor_copy)(out=out, in_=in_)

    with tile.TileContext(nc) as tc, ExitStack() as _es:
        _p = lambda **kw: _es.enter_context(tc.tile_pool(**kw))
        consts = _p(name="consts", bufs=1)
        xtp = _p(name="xtp", bufs=3)
        xntp = _p(name="xntp", bufs=2)
        xnp = _p(name="xnp", bufs=3)
        hp = _p(name="hp", bufs=2)
        htp = _p(name="htp", bufs=2)
        sup = _p(name="sup", bufs=1)
        newt = _p(name="newt", bufs=2)
        smalls = _p(name="smalls", bufs=4)
        psbp = _p(name="psbp", bufs=2)
        tsqp = _p(name="tsqp", bufs=2)
        outp = _p(name="outp", bufs=2)
        # PSUM budget (16KB/partition): pt 2x1KB + big512 3x2KB + ns 2x0.5KB
        # + hq 3x2KB = 15KB
        pt = _p(name="pt", bufs=2, space="PSUM")
        pacc = _p(name="pacc", bufs=3, space="PSUM")
        nsp = _p(name="nsp", bufs=1, space="PSUM")
        hq = _p(name="hq", bufs=2, space="PSUM")
        if True:
            # ---------- constants ----------
            ident_sb = consts.tile([128, 128], F32, tag="ident")
            nc.gpsimd.dma_start(out=ident_sb, in_=identd[:, :])
            identh_sb = consts.tile([128, 128], F16, tag="identh")
            nc.gpsimd.dma_start(out=identh_sb, in_=identd[:, :])
            p16_sb = consts.tile([128, 128], F32, tag="p16")
            nc.gpsimd.dma_start(out=p16_sb, in_=p16d[:, :])
            mask8_sb = consts.tile([128, 4, 2, 4], F32, tag="mask8")
            nc.gpsimd.dma_start(out=mask8_sb, in_=mask8d[:, :, :, :])
            oq8_sb = consts.tile([128, 4, 2, 4], F16, tag="oq8")
            nc.vector.tensor_scalar_mul(
                oq8_sb.rearrange("p a b c -> p (a b c)"),
                mask8_sb.rearrange("p a b c -> p (a b c)"), 1.0 / (Q - 1))
            ones_sb = consts.tile([128, 1], F32, tag="ones")
            nc.vector.memset(ones_sb, 1.0)
            zer_sb = consts.tile([128, 256], F16, tag="zer")
            nc.vector.memset(zer_sb, 0.0)

            rtt, wtt = {}, {}
            for m, (c, sp) in enumerate(MEMBERS):
                cc_n = c // 128
                wtt[m] = consts.tile([128, cc_n, Q], F16, tag=f"wt{m}",
                                     name=f"wt{m}")
                nc.gpsimd.dma_start(
                    out=wtt[m],
                    in_=wt[m][:, :].rearrange("(k p) q -> p k q", p=128))
                if m in rts:
                    nk = rts[m].shape[0] // 128
                    rtt[m] = consts.tile([128, nk, WH], F16, tag=f"rt{m}",
                                         name=f"rt{m}")
                    nc.gpsimd.dma_start(
                        out=rtt[m],
                        in_=rtd[m][:, :].rearrange("(k p) w -> p k w", p=128))

            ct_all, gsup = [], []
            for spr in range(2):
                t = sup.tile([128, DTOT], F16, tag=f"ct{spr}", name=f"ct{spr}")
                ct_all.append(t)
                g = sup.tile([128, 128], F32, tag=f"gsup{spr}",
                             name=f"gsup{spr}")
                nc.vector.memset(g, 0.0)
                gsup.append(g)

            m2_sb = [None, None]
            r_sb = [None, None]

            # ============ phase 1+2, super-outer =============
            for spr in range(2):
                for m in MORDER:
                    c, sp = MEMBERS[m]
                    uv = sp * sp
                    cc_n = c // 128
                    kch = _chunks(uv)
                    nch = _chunks(c, 512)

                    xnt0, xnt1 = {}, {}
                    for si in range(2):
                        s = 2 * spr + si
                        x0t = xntp.tile([128, c], F16, tag=f"xnt0_{si}",
                                        name=f"xnt0_{si}")
                        x1t = xntp.tile([68, c], F16, tag=f"xnt1_{si}",
                                        name=f"xnt1_{si}")
                        xnt0[si], xnt1[si] = x0t, x1t

                        if m == 2:
                            # identity resize: XrT comes straight from DRAM
                            nc.gpsimd.dma_start(out=x0t,
                                                in_=xin[m][s, 0:128, :])
                            nc.gpsimd.dma_start(out=x1t[:68, :],
                                                in_=xin[m][s, 128:196, :])
                        else:
                            # xt = x^T [uv(part), c] fp16, direct DMA-cast
                            xt = xtp.tile([128, len(kch), c], F16,
                                          tag=f"xt{m}", name=f"xt{m}")
                            nfull = uv // 128
                            rem = uv - nfull * 128
                            if nfull:
                                nc.gpsimd.dma_start(
                                    out=xt[:, :nfull, :],
                                    in_=xin[m][s, 0:nfull * 128, :].rearrange(
                                        "(k p) c -> p k c", p=128))
                            if rem:
                                nc.gpsimd.dma_start(
                                    out=xt[:rem, nfull, :],
                                    in_=xin[m][s, nfull * 128:uv, :])
                            for ni, (no, nsz) in enumerate(nch):
                                for Mo, Msz, dst in ((0, 128, x0t),
                                                     (128, 68, x1t)):
                                    pr = pacc.tile([128, 512], F32, tag="pacc")
                                    ks = nzk[m][0 if Mo == 0 else 1]
                                    for idx, ki in enumerate(ks):
                                        ko, ksz = kch[ki]
                                        nc.tensor.matmul(
                                            pr[:Msz, :nsz],
                                            lhsT=rtt[m][:ksz, ki, Mo:Mo + Msz],
                                            rhs=xt[:ksz, ki, no:no + nsz],
                                            start=(idx == 0),
                                            stop=(idx == len(ks) - 1))
                                    _pcopy(out=dst[:Msz, no:no + nsz],
                                           in_=pr[:Msz, :nsz])

                        # zscore along free dim
                        for tl, psz in ((x0t, 128), (x1t, 68)):
                            gs = _chunks(c, 512)
                            st = smalls.tile([128, len(gs), 6], F32, tag="bnst")
                            for gi, (go, gln) in enumerate(gs):
                                nc.vector.bn_stats(out=st[:psz, gi, :],
                                                   in_=tl[:psz, go:go + gln])
                            mv = smalls.tile([128, 2], F32, tag="mv")
                            nc.vector.bn_aggr(out=mv[:psz], in_=st[:psz])
                            sd = smalls.tile([128, 1], F32, tag="sd")
                            nc.scalar.activation(out=sd[:psz], in_=mv[:psz, 1:2],
                                                 func=AF.Sqrt,
                                                 scale=c / (c - 1.0))
                            sc = smalls.tile([128, 1], F32, tag="sc")
                            nc.vector.reciprocal(out=sc[:psz], in_=sd[:psz])
                            bi = smalls.tile([128, 1], F32, tag="bi")
                            nc.vector.tensor_scalar(
                                out=bi[:psz], in0=mv[:psz, 0:1],
                                scalar1=sc[:psz], scalar2=-1.0,
                                op0=ALU.mult, op1=ALU.mult)
                            nc.gpsimd.tensor_scalar(
                                out=tl[:psz], in0=tl[:psz],
                                scalar1=sc[:psz], scalar2=bi[:psz],
                                op0=ALU.mult, op1=ALU.add)

                    # ---- H = sigmoid(W @ Xn) over the sample pair ----
                    hps = hq.tile([16, 2 * WH], F32, tag="hq")
                    for c2 in range(0, cc_n, 2):
                        cn = min(2, cc_n - c2)
                        xn = xnp.tile([128, 2, 2 * WH], F16, tag="xn")
                        for ci in range(cn):
                            ps = pt.tile([128, 512], F16, tag="pt")
                            cc = c2 + ci
                            for si in range(2):
                                off = si * WH
                                nc.tensor.transpose(
                                    ps[:128, off:off + 128],
                                    xnt0[si][:, cc * 128:(cc + 1) * 128],
                                    identh_sb)
                                nc.tensor.transpose(
                                    ps[:128, off + 128:off + 196],
                                    xnt1[si][:68, cc * 128:(cc + 1) * 128],
                                    identh_sb[:68, :68])
                            _pcopy(out=xn[:, ci, :], in_=ps[:, 0:2 * WH])
                        for ci in range(cn):
                            cc = c2 + ci
                            nc.tensor.matmul(
                                hps, lhsT=wtt[m][:, cc, :],
                                rhs=xn[:, ci, :],
                                start=(cc == 0), stop=(cc == cc_n - 1))
                    hsb = hp.tile([16, 2, WH], F16, tag="hsb")
                    nc.scalar.activation(
                        out=hsb.rearrange("p a b -> p (a b)"),
                        in_=hps, func=AF.Sigmoid)

                    htbig = {}
                    for si in range(2):
                        b = 4 * si + m
                        bo = 16 * b
                        htb = htp.tile([128, 2, 128], F16, tag=f"htbig{m}")
                        if spr == 0:
                            # ring-2 per member: spr 1 reuses the same buffer
                            # and overwrites the same 16-column block, so the
                            # zero background survives
                            nc.gpsimd.tensor_copy(
                                out=htb.rearrange("p a b -> p (a b)"),
                                in_=zer_sb)
                        pht = pt.tile([128, 512], F16, tag="pt")
                        nc.tensor.transpose(pht[:128, 0:Q],
                                            hsb[:Q, si, 0:128],
                                            identh_sb[:Q, :Q])
                        nc.tensor.transpose(pht[:68, Q:2 * Q],
                                            hsb[:Q, si, 128:196],
                                            identh_sb[:Q, :Q])
                        nc.vector.tensor_copy(
                            out=htb[:, 0, bo:bo + Q],
                            in_=pht[:128, 0:Q])
                        nc.vector.tensor_copy(
                            out=htb[:68, 1, bo:bo + Q],
                            in_=pht[:68, Q:2 * Q])
                        htbig[si] = htb
                        # G block accumulation into SBUF accumulator
                        gp = nsp.tile([128, 128], F32, tag="ns")
                        for wi, wsz in ((0, 128), (1, 68)):
                            nc.tensor.matmul(gp,
                                             lhsT=htb[:wsz, wi, :],
                                             rhs=htb[:wsz, wi, :],
                                             start=(wi == 0), stop=(wi == 1))
                        nc.vector.tensor_tensor(out=gsup[spr], in0=gsup[spr],
                                                in1=gp, op=ALU.add)
                    # C^T for both blocks of the pair
                    for ni, (no, nsz) in enumerate(nch):
                        cps = pacc.tile([128, 512], F32, tag="pacc")
                        idx = 0
                        for si in range(2):
                            for wi, wsz, xs in ((0, 128, xnt0[si]),
                                                (1, 68, xnt1[si])):
                                nc.tensor.matmul(
                                    cps[:, :nsz],
                                    lhsT=htbig[si][:wsz, wi, :],
                                    rhs=xs[:wsz, no:no + nsz],
                                    start=(idx == 0), stop=(idx == 3))
                                idx += 1
                        _pcopy(out=ct_all[spr][:, OFFS[m] + no:OFFS[m] + no +
                                               nsz],
                               in_=cps[:, :nsz])

                # ---- phase 2 for this super: Newton-Schulz ----
                # fp16 iterations (PE 1 cyc/row) + one fp32 polish iteration;
                # the polish contracts fp16 iteration error quadratically.
                g = gsup[spr]
                gh = newt.tile([128, 128], F16, tag="gh")
                nc.vector.tensor_copy(out=gh, in_=g)
                sq = newt.tile([128, 128], F32, tag="sq")
                nc.vector.tensor_mul(sq, g, g)
                rs = newt.tile([128, 1], F32, tag="rs")
                nc.vector.tensor_reduce(out=rs, in_=sq,
                                        axis=mybir.AxisListType.X, op=ALU.add)
                bps = nsp.tile([128, 128], F32, tag="ns")
                nc.tensor.matmul(bps[:128, 0:1], lhsT=p16_sb, rhs=rs,
                                 start=True, stop=True)
                bf = newt.tile([128, 1], F32, tag="bf")
                nc.scalar.activation(out=bf, in_=bps[:128, 0:1], func=AF.Sqrt)
                al = newt.tile([128, 1], F32, tag="al")
                nc.vector.reciprocal(out=al, in_=bf)
                xh = newt.tile([128, 128], F16, tag="xh")
                nc.vector.tensor_scalar_mul(xh, ident_sb, al)
                xps = None
                for it in range(NEWTON_ITERS):
                    yps = nsp.tile([128, 128], F32, tag="ns")
                    nc.tensor.matmul(yps, lhsT=gh, rhs=xh,
                                     start=True, stop=True)
                    zh = newt.tile([128, 128], F16, tag="zh")
                    nc.vector.scalar_tensor_tensor(
                        out=zh, in0=ident_sb, scalar=2.0,
                        in1=yps, op0=ALU.mult, op1=ALU.subtract)
                    xps = nsp.tile([128, 128], F32, tag="ns")
                    nc.tensor.matmul(xps, lhsT=xh, rhs=zh,
                                     start=True, stop=True)
                    if it < NEWTON_ITERS - 1:
                        xh = newt.tile([128, 128], F16, tag="xh")
                        nc.scalar.copy(out=xh, in_=xps)
                x32 = newt.tile([128, 128], F32, tag="x32")
                nc.scalar.copy(out=x32, in_=xps)
                # symmetrize: fp16 iterations accumulate an antisymmetric
                # error component the polish cannot contract
                xtps = nsp.tile([128, 128], F32, tag="ns")
                nc.tensor.transpose(xtps, x32, ident_sb)
                xsum = newt.tile([128, 128], F32, tag="xsum")
                nc.vector.tensor_tensor(out=xsum, in0=x32, in1=xtps,
                                        op=ALU.add)
                x32 = newt.tile([128, 128], F32, tag="x32")
                nc.scalar.mul(out=x32, in_=xsum, mul=0.5)
                for _pol in range(2):
                    yp2 = nsp.tile([128, 128], F32, tag="ns")
                    nc.tensor.matmul(yp2, lhsT=g, rhs=x32, start=True,
                                     stop=True)
                    z32 = newt.tile([128, 128], F32, tag="z32")
                    nc.vector.scalar_tensor_tensor(
                        out=z32, in0=ident_sb, scalar=2.0,
                        in1=yp2, op0=ALU.mult, op1=ALU.subtract)
                    xp2 = nsp.tile([128, 128], F32, tag="ns")
                    nc.tensor.matmul(xp2, lhsT=x32, rhs=z32, start=True,
                                     stop=True)
                    x32 = newt.tile([128, 128], F32, tag="x32")
                    nc.scalar.copy(out=x32, in_=xp2)
                xf = x32
                mps = nsp.tile([128, 128], F32, tag="ns")
                nc.tensor.matmul(mps, lhsT=xf, rhs=xf,
                                 start=True, stop=True)
                m2t = sup.tile([128, 128], F16, tag=f"m2_{spr}",
                               name=f"m2_{spr}")
                nc.vector.tensor_copy(out=m2t, in_=mps)
                m2_sb[spr] = m2t
                rps = nsp.tile([128, 128], F32, tag="ns")
                nc.tensor.matmul(rps[:128, 0:1], lhsT=xf, rhs=ones_sb,
                                 start=True, stop=True)
                rt_ = sup.tile([128, 1], F32, tag=f"r_{spr}", name=f"r_{spr}")
                nc.vector.tensor_copy(out=rt_, in_=rps[:128, 0:1])
                r_sb[spr] = rt_

            # r4[m][spr] = mask8[m, spr] * r_super(spr)
            r4 = []
            for m in range(4):
                t = smalls.tile([128, 2, 4], F16, tag="r4", name=f"r4_{m}")
                nc.vector.tensor_scalar_mul(t[:, 0, :],
                                            mask8_sb[:, m, 0, :], r_sb[0])
                nc.vector.tensor_scalar_mul(t[:, 1, :],
                                            mask8_sb[:, m, 1, :], r_sb[1])
                r4.append(t)

            # ================= phase 3: variance readout =================
            for m, (c, sp) in enumerate(MEMBERS):
                for ni, (no, nsz) in enumerate(_chunks(c, 512)):
                    g0 = OFFS[m] + no
                    psbs = []
                    for spr in range(2):
                        dfp = pacc.tile([128, 512], F32, tag="pacc")
                        nc.tensor.matmul(
                            dfp[:, :nsz], lhsT=m2_sb[spr],
                            rhs=ct_all[spr][:, g0:g0 + nsz],
                            start=True, stop=True)
                        psb = psbp.tile([128, 512], F16, tag="psb")
                        nc.vector.tensor_mul(psb[:, :nsz],
                                             ct_all[spr][:, g0:g0 + nsz],
                                             dfp[:, :nsz])
                        psbs.append(psb)
                    qps = hq.tile([4, 512], F32, tag="hq")
                    tps = hq.tile([4, 512], F32, tag="hq")
                    for spr in range(2):
                        nc.tensor.matmul(
                            qps[:, :nsz],
                            lhsT=oq8_sb[:, m, spr, :],
                            rhs=psbs[spr][:, :nsz],
                            start=(spr == 0), stop=(spr == 1))
                        nc.tensor.matmul(
                            tps[:, :nsz], lhsT=r4[m][:, spr, :],
                            rhs=ct_all[spr][:, g0:g0 + nsz],
                            start=(spr == 0), stop=(spr == 1))
                    tsq = tsqp.tile([S, 512], F32, tag="tsq")
                    nc.scalar.square(out=tsq[:, :nsz], in_=tps[:S, :nsz])
                    ot = outp.tile([S, 512], F32, tag="out")
                    nc.vector.scalar_tensor_tensor(
                        out=ot[:, :nsz],
                        in0=tsq[:, :nsz], scalar=-1.0 / ((Q - 1) * Q),
                        in1=qps[:S, :nsz], op0=ALU.mult, op1=ALU.add)
                    nc.sync.dma_start(out=outd[:, g0:g0 + nsz],
                                      in_=ot[:, :nsz])

    nc.finalize()
    return nc


_CACHE = {}


def _in_maps(inputs):
    rts, ident_np, p16_np, mask8_np = _consts()
    xs = [np.asarray(inputs[f"x{m}"]) for m in range(4)]
    ws = [np.asarray(inputs[f"W{m}"]) for m in range(4)]
    in_maps = []
    for i in range(NCORES):
        im = {"ident": ident_np, "p16": p16_np, "mask8": mask8_np}
        for m, (c, sp) in enumerate(MEMBERS):
            xi = xs[m][S * i:S * (i + 1)].reshape(S, c, sp * sp)
            im[f"x{m}"] = np.ascontiguousarray(
                xi.transpose(0, 2, 1), np.float16)      # [S, uv, c] fp16
            im[f"wt{m}"] = np.ascontiguousarray(ws[m].T, np.float32)
            if m in rts:
                im[f"rt{m}"] = rts[m]
        in_maps.append(im)
    return in_maps


def kernel(x0, x1, x2, x3, W0, W1, W2, W3):
    if "nc" not in _CACHE:
        _CACHE["nc"] = _build_program()
    nc = _CACHE["nc"]
    in_maps = _in_maps(dict(x0=x0, x1=x1, x2=x2, x3=x3,
                            W0=W0, W1=W1, W2=W2, W3=W3))
    res = run_bass_kernel_spmd(nc, in_maps, list(range(NCORES)))
    return np.concatenate([r["out"] for r in res.results], axis=0)
